# revision 59
# baseline (speedup 1.0000x reference)
"""Trainium2 Bass kernel for causal self-attention with doubled rotary.

Full-input contract: kernel(**inputs) takes the complete tensors
(x [4,2048,2048], wq/wk/wv/wo [2048,2048]) and returns [4,2048,2048] fp32.

Sharding: 8 cores = 4 batch elements x 2 head-halves (8 heads each).
Each core computes a partial output projection (its heads' columns of wo);
the host sums the two partials per batch element.

All matmul operands are bf16 (fp8 exceeds the error budget on every path —
measured 2.8-4.7e-2 vs the 2e-2 gate; all-bf16 lands at ~3.6e-3). bf16
halves DMA bytes vs fp32r and enables FWL weight loads.

Per-core structure (engine queues execute strictly in emission order, and
the shared DMA engines serve queue heads round-robin — both facts shape
every choice below):
  - ramp: ALL phase-0 loads ride the sync queue in exact consumption
    order (x-p0/wq quarters interleaved, then wk, then wv) so the global
    DMA service order matches sweep A's chain order.
  - sweep A: per x panel, q/k projections of group 0 + V (8 heads, low
    column half). The doubled-angle rotary (R(t)^2 == R(2t)) is split:
    half-swap DMAs are issued right after the panel's qk chains, the DVE
    ops lag one panel (the vector engine must never park on a swap).
  - the V high-half sweep is folded into segments 1-2 as zip filler (no
    standalone sweep, no boundary stall; 8MB of x re-reads spread out).
  - segments 1..3 in ONE proportional zip: projection+rotary of group g
    with attention of head pair g-1. Weight prefetch for g+1 is emitted
    at the START of segment g (slots free exactly then; ~a full segment
    of DMA lead); the x panel-0 prefetch mid-segment.
  - attention is computed transposed (ST[s,t]) so exp(ST) feeds the PV
    matmul directly with v stationary. Chunks run diagonal-first with a
    one-chunk QK/PV software-pipeline skew: every PV's exp latency is
    hidden by the next chunk's QKs — the attention stream is
    self-covering. Finalize defers one chunk into the next panel.
  - softmax denominator: the exp chunks accumulate into two fp16 panel
    accumulators on SEPARATE engines (hl0: vector, hl1: gpsimd); ONE
    all-ones matmul per (head, panel) does the partition reduce +
    broadcast.
  - y stays resident in SBUF; the output projection reads it directly,
    zipped with the last attention pair's panels. Output is written bf16
    (host upcasts and sums the two partials) to halve the output DMA.
  - PSUM: 4 banks for attention scores (QK/PV skew + denominator), 2 for
    PV accumulation, 2 for projections / output tiles.
"""

import os
import sys

for _p in ("/opt/trn_rl_repo", "/root/.axon_site/_ro/trn_rl_repo"):
    if os.path.isdir(_p) and _p not in sys.path:
        sys.path.insert(0, _p)

import numpy as np

import concourse.bass as bass
import concourse.mybir as mybir
from concourse import bacc
from concourse.bass import ds
from concourse.tile import TileContext
from concourse.bass_utils import run_bass_kernel_spmd

F32 = mybir.dt.float32
F32R = mybir.dt.float32r
BF16 = mybir.dt.bfloat16
FP16 = mybir.dt.float16

P = 128          # partitions / head dim
T = 2048         # sequence length
E = 2048         # embedding dim
B = 4
HPC = 8          # heads per core
D = 128          # head dim
PAN = 512        # panel width (PSUM bank limit for fp32)
NPAN = T // PAN  # 4
EO = E // P      # 16 contraction chunks for projections
EQ = 4           # eo chunks per input-DMA quarter
NGRP = 4         # head pairs per core
NCH = T // P     # 16 s-chunks (also v t-tiles)
SCALE = 1.0 / float(np.sqrt(D))
NEG = -1.0e9

ADD = mybir.AluOpType.add
MULT = mybir.AluOpType.mult
EXP = mybir.ActivationFunctionType.Exp


def _zip_emit(*lists):
    """Emit thunks from several lists round-robin, proportionally."""
    lists = [list(l) for l in lists if l]
    if not lists:
        return
    total = max(len(l) for l in lists)
    idx = [0.0] * len(lists)
    step = [len(l) / total for l in lists]
    for _ in range(total):
        for li, l in enumerate(lists):
            idx[li] += step[li]
            while idx[li] >= 1.0 and l:
                l.pop(0)()
                idx[li] -= 1.0
    for l in lists:
        for f in l:
            f()


class Ctx:
    pass


def _dma_quarters(nc, dst, src_re, eng=None, parts=4):
    """Split a [P, EO, W] load into `parts` DMAs. Quartering helps only
    when the consumer is waiting (ramp); elsewhere fewer DMAs conserve
    the shared DMA-completion semaphore ring (~19 ids across all queues
    — each extra DMA risks a recycle stall on the 19-back semaphore)."""
    eng = eng if eng is not None else nc.sync
    step = EO // parts
    for qq in range(parts):
        eng.dma_start(
            dst[:, ds(qq * step, step), :], src_re[:, ds(qq * step, step), :]
        )


def build_program():
    nc = bacc.Bacc()
    cx = Ctx()
    cx.nc = nc

    cx.xT = nc.declare_dram_parameter("xT", [E, T], BF16, isOutput=False)
    # wq/wk use a group-major, eo-paired host layout [g, eo2, p, two, d]
    # so each group load has 1KB contiguous runs (512B runs pay a 2x DMA
    # latency multiplier)
    cx.wqT = nc.declare_dram_parameter(
        "wqT", [NGRP * (E // 2), 4 * D], BF16, isOutput=False)
    cx.wkT = nc.declare_dram_parameter(
        "wkT", [NGRP * (E // 2), 4 * D], BF16, isOutput=False)
    cx.wvT = nc.declare_dram_parameter("wvT", [E, HPC * D], BF16, isOutput=False)
    cx.woT = nc.declare_dram_parameter("woT", [HPC * D, E], BF16, isOutput=False)
    cx.cos2 = nc.declare_dram_parameter("cos2", [P, T], FP16, isOutput=False)
    cx.sin2 = nc.declare_dram_parameter("sin2", [P, T], FP16, isOutput=False)
    cx.mask = nc.declare_dram_parameter("mask", [P, P], BF16, isOutput=False)
    cx.out = nc.declare_dram_parameter("out", [E, T], BF16, isOutput=True)

    with TileContext(nc) as tc:
        cx.tc = tc
        with tc.tile_pool(name="const", bufs=1) as cpool:
            om_f = cpool.tile([P, P], F32, tag="om_f")
            nc.vector.memset(om_f, 1.0)
            cx.onesmat = cpool.tile([P, P], FP16, tag="onesmat")
            nc.scalar.copy(cx.onesmat, om_f)
            cx.mk = cpool.tile([P, P], BF16, tag="mk")

            with (
                tc.tile_pool(name="ex", bufs=6) as expool,
                tc.tile_pool(name="acc", bufs=2) as accpool,
                tc.tile_pool(name="dn", bufs=1) as dnpool,
                tc.tile_pool(name="qk", bufs=2) as qkpool,
                tc.tile_pool(name="vp", bufs=1) as vpool,
                tc.tile_pool(name="yp", bufs=1) as ypool,
                tc.tile_pool(name="psS", bufs=3, space="PSUM") as psS,
                tc.tile_pool(name="psY", bufs=3, space="PSUM") as psY,
            ):
                cx.expool, cx.accpool, cx.dnpool = expool, accpool, dnpool
                cx.qkpool = qkpool
                cx.v_sb = vpool.tile([P, NCH, HPC * D], BF16, tag="v")
                cx.y_sb = ypool.tile([P, HPC, T], BF16, tag="y")
                cx.psS, cx.psY = psS, psY
                cx.qkv = {}      # g -> (qT, kT)
                cx._w = {}       # g -> (wq_sb, wk_sb)
                cx._pstate = {}  # g -> {xj: xp tile}

                with (
                    tc.tile_pool(name="tab", bufs=1) as tabpool,
                    tc.tile_pool(name="xp", bufs=2) as xpool,
                    tc.tile_pool(name="wqk", bufs=2) as wqkpool,
                    tc.tile_pool(name="wv", bufs=1) as wvpool,
                    tc.tile_pool(name="rot", bufs=1) as rotpool,
                    tc.tile_pool(name="sw", bufs=2) as swpool,
                    tc.tile_pool(name="psP", bufs=2, space="PSUM") as psP,
                ):
                    cx.xpool, cx.wqkpool, cx.wvpool = xpool, wqkpool, wvpool
                    cx.rotpool, cx.swpool, cx.psP = rotpool, swpool, psP
                    cx.tabpool = tabpool

                    # --- phase 0 ramp: quarter loads on three queues ---
                    _phase0_loads(cx)

                    # sweep A: per panel qk(g0) + v(half0). Chains run in
                    # LOCKSTEP pairs (two psum banks, quarter-interleaved)
                    # in arrival order x/wq -> wk -> wv, so the PE gets
                    # ~2x the work per arriving DMA quarter during the
                    # bandwidth-bound ramp. Each panel's rotary is spread
                    # into the NEXT panel's PE thunks.
                    state0 = cx._pstate.setdefault(0, {})
                    pending_rot = []
                    for xj in range(NPAN):
                        if xj + 1 < NPAN:
                            _load_panel(cx, xj + 1, state0)()
                        if xj < 2:
                            # DMA-bound panels: lockstep pairs
                            pe = [
                                _qk_lockstep(cx, 0, xj, 0),
                                _qk_lockstep(cx, 0, xj, 1),
                                _v_lockstep(cx, state0, xj, 0, 0),
                                _v_lockstep(cx, state0, xj, 1, 0),
                            ]
                        else:
                            # data resident: single chains pipeline the
                            # psP ring without pair-boundary cast waits
                            pe = [
                                _qk_group(cx, 0, xj, 0, 0),
                                _qk_group(cx, 0, xj, 0, 1),
                                _v_group(cx, state0, xj, 0, 0),
                                _qk_group(cx, 0, xj, 1, 0),
                                _v_group(cx, state0, xj, 1, 0),
                                _qk_group(cx, 0, xj, 1, 1),
                                _v_group(cx, state0, xj, 2, 0),
                                _v_group(cx, state0, xj, 3, 0),
                            ]
                        for f in pe:
                            f()
                            if pending_rot:
                                pending_rot.pop(0)()
                        if xj == 0:
                            _load_tables(cx)
                        for f in _rot_swap_thunks(cx, 0, xj):
                            f()
                        pending_rot = _rot_dve_thunks(cx, 0, xj)

                    # prefetches for segment 1 (wv half 1 replaces half 0
                    # in place — its slot frees exactly at sweep A's end)
                    for f in _prefetch_weights(cx, 1):
                        f()
                    _load_wv_half(cx, 1)()
                    _prefetch_panel0(cx, 1)()

                    # The v-half1 sweep is folded into segments 1-2 as
                    # extra PE filler: its x reloads and chains spread
                    # across the zip (no standalone sweep-B boundary),
                    # and attention gets more matmul cover. attn(2) first
                    # reads half 1 in segment 3 — complete by then.
                    vstate = {}

                    def vload(xj):
                        return _load_panel(cx, xj, vstate, parts=2)

                    def vchain(xj, tt):
                        return _v_group(cx, vstate, xj, tt, 1)

                    extras = {
                        1: [[vload(0)],
                            [vchain(0, 0), vchain(0, 1), vload(1)],
                            [vchain(0, 2), vchain(0, 3), vchain(1, 0)],
                            [vchain(1, 1), vchain(1, 2), vchain(1, 3)]],
                        2: [[vload(2)],
                            [vchain(2, 0), vchain(2, 1), vload(3)],
                            [vchain(2, 2), vchain(2, 3), vchain(3, 0)],
                            [vchain(3, 1), vchain(3, 2), vchain(3, 3)]],
                        3: [[], [], [], []],
                    }

                    # phases 1..3 merged into ONE proportional zip so the
                    # scheduler always has projection matmuls available to
                    # hide the exp (ACT) latency of attention chunks.
                    # attn(3, jp=0) joins the zip tail so the final
                    # outproj section starts with oproj[0] ready.
                    projall = []
                    attnall = []
                    for g in range(1, NGRP):
                        if g + 1 < NGRP:
                            projall += _prefetch_weights(cx, g + 1)
                        segment, pending_rot = _proj_thunks(
                            cx, g, pending_rot, extras[g]
                        )
                        projall += segment
                        attnall += _attn_thunks(cx, g - 1)
                    projall += pending_rot
                    attnall += _attn_thunks(cx, NGRP - 1, only_jp=0)
                    _zip_emit(projall, attnall)

                with (
                    tc.tile_pool(name="wo", bufs=1) as wopool,
                    tc.tile_pool(name="ob", bufs=3) as opool,
                    tc.tile_pool(name="psO", bufs=2, space="PSUM") as psO,
                ):
                    cx.opool, cx.psO = opool, psO
                    cx.wo_sb = wopool.tile([P, HPC, E], BF16, tag="wo")
                    # halved, low e-columns first, so the first outproj
                    # e-tiles only wait on the first 2MB
                    for qq in range(2):
                        nc.sync.dma_start(
                            cx.wo_sb[:, :, ds(qq * (E // 2), E // 2)],
                            cx.woT.rearrange("(c p) e -> p c e", p=P)[
                                :, :, ds(qq * (E // 2), E // 2)
                            ],
                        )
                    # attn(3, jp=0) already ran in the main zip, so every
                    # remaining attention panel has an outproj panel of
                    # pure matmul work to hide its exp latency.
                    oproj = [_outproj_thunks(cx, jp) for jp in range(NPAN)]
                    for jp in range(1, NPAN):
                        _zip_emit(
                            _attn_thunks(cx, NGRP - 1, only_jp=jp),
                            oproj[jp - 1],
                        )
                    for f in oproj[NPAN - 1]:
                        f()

    nc.finalize()
    return nc


def _load_panel(cx, xj, state, eng=None, parts=2):
    def f():
        xp = cx.xpool.tile([P, EO, PAN], BF16, tag="xp")
        _dma_quarters(
            cx.nc, xp,
            cx.xT.rearrange("(eo p) t -> p eo t", p=P)[:, :, ds(xj * PAN, PAN)],
            eng=eng,
            parts=parts,
        )
        state[xj] = xp
    return f


def _phase0_loads(cx):
    """Phase-0 preamble. The shared DMA engines serve queue heads
    round-robin, so global transfer ORDER (not queue parallelism) sets
    the ramp: EVERYTHING rides the sync queue in exact consumption
    order of sweep A's chains — x-p0/wq quarters interleaved (first two
    chains), then wk (chains 3-4, ~16us in), then wv (v chains,
    ~23us in). Any second queue would steal round-robin bandwidth from
    the head of this sequence."""
    nc = cx.nc
    state = cx._pstate.setdefault(0, {})
    xp = cx.xpool.tile([P, EO, PAN], BF16, tag="xp")
    state[0] = xp
    xsrc = cx.xT.rearrange("(eo p) t -> p eo t", p=P)
    wq_sb = cx.wqkpool.tile([P, EO // 2, 4 * D], BF16, tag="wq")
    wqsrc = cx.wqT.rearrange("(g eo2 p) dd -> p g eo2 dd", g=NGRP, p=P)
    for qq in range(EO // EQ):
        nc.sync.dma_start(
            xp[:, ds(qq * EQ, EQ), :], xsrc[:, ds(qq * EQ, EQ), ds(0, PAN)]
        )
        nc.sync.dma_start(
            wq_sb[:, ds(qq * 2, 2), :], wqsrc[:, 0, ds(qq * 2, 2), :]
        )
    wk_sb = cx.wqkpool.tile([P, EO // 2, 4 * D], BF16, tag="wk")
    wksrc = cx.wkT.rearrange("(g eo2 p) dd -> p g eo2 dd", g=NGRP, p=P)
    for hh in range(2):
        nc.sync.dma_start(
            wk_sb[:, ds(hh * 4, 4), :], wksrc[:, 0, ds(hh * 4, 4), :]
        )
    wv_sb = cx.wvpool.tile([P, EO, HPC * D // 2], BF16, tag="wv")
    _dma_quarters(
        nc, wv_sb,
        cx.wvT.rearrange("(eo p) d -> p eo d", p=P)[
            :, :, ds(0, HPC * D // 2)
        ],
        eng=nc.sync,
        parts=4,
    )
    cx._wv = wv_sb
    qT = cx.qkpool.tile([P, 2, T], BF16, tag="qT")
    kT = cx.qkpool.tile([P, 2, T], BF16, tag="kT")
    cx.qkv[0] = (qT, kT)
    cx._w[0] = (wq_sb, wk_sb)


def _load_tables(cx):
    """Rotary tables + causal mask. Emitted after panel 0's PE thunks and
    on the SYNC queue so their 1MB rides BEHIND the ramp-critical loads
    in the global DMA service order (on gpsimd they issued immediately
    and stole round-robin bandwidth from the first x/wq transfers); they
    are needed only by rot(0, p0) at ~35us."""
    nc = cx.nc
    cx.c2 = cx.tabpool.tile([P, T], FP16, tag="c2")
    nc.sync.dma_start(cx.c2, cx.cos2[:, :])
    cx.s2 = cx.tabpool.tile([P, T], FP16, tag="s2")
    nc.sync.dma_start(cx.s2, cx.sin2[:, :])
    nc.sync.dma_start(cx.mk, cx.mask[:, :])


def _load_wv_half(cx, half):
    def f():
        wv_sb = cx.wvpool.tile([P, EO, HPC * D // 2], BF16, tag="wv")
        _dma_quarters(
            cx.nc, wv_sb,
            cx.wvT.rearrange("(eo p) d -> p eo d", p=P)[
                :, :, ds(half * HPC * D // 2, HPC * D // 2)
            ],
            eng=cx.nc.scalar,
        )
        cx._wv = wv_sb
    return f


def _v_group(cx, state, xj, tt, half):
    """v for all 8 heads, one s-chunk, one 512-column half."""
    def f():
        nc = cx.nc
        xp = state[xj]
        ps = cx.psP.tile([P, PAN], F32, tag="psP")
        for eo in range(EO):
            nc.tensor.matmul(
                ps,
                lhsT=xp[:, eo, ds(tt * P, P)],
                rhs=cx._wv[:, eo, :],
                start=(eo == 0),
                stop=(eo == EO - 1),
            )
        nc.scalar.copy(
            cx.v_sb[:, xj * (PAN // P) + tt, ds(half * PAN, PAN)], ps
        )
    return f


def _qk_lockstep(cx, g, xj, wi):
    """Both hl chains of weight wi, quarter-interleaved on two psum
    banks: during the DMA-bound ramp each arriving x/w quarter feeds
    2x the PE work of a single chain."""
    nc = cx.nc

    def f():
        xp = cx._pstate[g][xj]
        w_sb = cx._w[g][wi]
        dst = cx.qkv[g][wi]
        ps = [cx.psP.tile([P, PAN], F32, tag="psP", name=f"psq{hl}")
              for hl in range(2)]
        for qq in range(EO // EQ):
            for hl in range(2):
                for eo in range(qq * EQ, (qq + 1) * EQ):
                    nc.tensor.matmul(
                        ps[hl],
                        lhsT=w_sb[:, eo // 2,
                                  ds((eo % 2) * 2 * D + hl * D, D)],
                        rhs=xp[:, eo, :],
                        start=(eo == 0),
                        stop=(eo == EO - 1),
                    )
        for hl in range(2):
            nc.vector.tensor_copy(dst[:, hl, ds(xj * PAN, PAN)], ps[hl])
    return f


def _v_lockstep(cx, state, xj, tp, half):
    """v for s-chunk pair (2*tp, 2*tp+1), quarter-interleaved."""
    nc = cx.nc

    def f():
        xp = state[xj]
        ps = [cx.psP.tile([P, PAN], F32, tag="psP", name=f"psv{i}")
              for i in range(2)]
        for qq in range(EO // EQ):
            for i in range(2):
                tt = 2 * tp + i
                for eo in range(qq * EQ, (qq + 1) * EQ):
                    nc.tensor.matmul(
                        ps[i],
                        lhsT=xp[:, eo, ds(tt * P, P)],
                        rhs=cx._wv[:, eo, :],
                        start=(eo == 0),
                        stop=(eo == EO - 1),
                    )
        for i in range(2):
            tt = 2 * tp + i
            nc.scalar.copy(
                cx.v_sb[:, xj * (PAN // P) + tt, ds(half * PAN, PAN)], ps[i]
            )
    return f


def _qk_group(cx, g, xj, wi, hl):
    """One q/k projection chain: head hl of weight wi over panel xj.
    Group 3's psum->SBUF casts go to scalar: segment 3 has no v-copy
    traffic there, while its vector queue (rot + biggest attention
    panels) is the recurring late-segment choke point."""
    nc = cx.nc

    def f():
        xp = cx._pstate[g][xj]
        w_sb = cx._w[g][wi]
        dst = cx.qkv[g][wi]
        ps = cx.psP.tile([P, PAN], F32, tag="psP")
        for eo in range(EO):
            nc.tensor.matmul(
                ps,
                lhsT=w_sb[:, eo // 2, ds((eo % 2) * 2 * D + hl * D, D)],
                rhs=xp[:, eo, :],
                start=(eo == 0),
                stop=(eo == EO - 1),
            )
        if g == NGRP - 1:
            nc.scalar.copy(dst[:, hl, ds(xj * PAN, PAN)], ps)
        else:
            nc.vector.tensor_copy(dst[:, hl, ds(xj * PAN, PAN)], ps)
    return f


def _prefetch_weights(cx, g):
    """Weight prefetch for group g: wq and wk quarters on sync. Emitted
    at the START of the previous segment — the wq/wk pool slots free
    exactly then, so the DMAs run with a full segment of lead. The
    scalar queue is left to exp and gpsimd to the rotary swaps."""
    nc = cx.nc

    def f():
        wq_sb = cx.wqkpool.tile([P, EO // 2, 4 * D], BF16, tag="wq")
        nc.sync.dma_start(
            wq_sb,
            cx.wqT.rearrange("(gg eo2 p) dd -> p gg eo2 dd", gg=NGRP, p=P)[
                :, g, :, :
            ],
        )
        wk_sb = cx.wqkpool.tile([P, EO // 2, 4 * D], BF16, tag="wk")
        nc.sync.dma_start(
            wk_sb,
            cx.wkT.rearrange("(gg eo2 p) dd -> p gg eo2 dd", gg=NGRP, p=P)[
                :, g, :, :
            ],
        )
        qT = cx.qkpool.tile([P, 2, T], BF16, tag="qT")
        kT = cx.qkpool.tile([P, 2, T], BF16, tag="kT")
        cx.qkv[g] = (qT, kT)
        cx._w[g] = (wq_sb, wk_sb)

    return [f]


def _prefetch_panel0(cx, g):
    """x panel-0 prefetch for group g (emitted mid-previous-segment)."""
    state = cx._pstate.setdefault(g, {})
    return _load_panel(cx, 0, state)


def _proj_thunks(cx, g, pending_rot, extras=None):
    """Thunks for group g's q/k projections. Panel xj's rotary thunks
    (DVE-only) are spread 1:1 into panel xj+1's qk chains so the PE
    always has matmul work between them; panel 3's rotary is returned
    as the pending list for the NEXT segment's first panel (attention
    only needs it for its jp=3 chunks, deep into that segment).
    Weights/qT/kT were allocated by _prefetch_weights in the previous
    segment; panel 0 was prefetched by _prefetch_panel0."""
    thunks = []
    state = cx._pstate.setdefault(g, {})
    rotq = list(pending_rot)

    for xj in range(NPAN):
        if xj + 1 < NPAN:
            thunks.append(_load_panel(cx, xj + 1, state))
            if xj + 2 == NPAN and g + 1 < NGRP:
                thunks.append(_prefetch_panel0(cx, g + 1))
        for wi in range(2):
            for hl in range(2):
                thunks.append(_qk_group(cx, g, xj, wi, hl))
                if rotq:
                    thunks.append(rotq.pop(0))
        thunks += _rot_swap_thunks(cx, g, xj)
        if extras is not None:
            thunks += extras[xj]
        rotq = _rot_dve_thunks(cx, g, xj)
    return thunks, rotq


def _rot_swap_thunks(cx, g, jp):
    """Half-swap DMAs for the rotary of group g, panel jp — batched
    across both heads (2 DMAs per tensor). Emitted right after the
    panel's qk chains so the transfers complete a full panel before the
    DVE ops (in _rot_dve_thunks) reach the vector queue head — the
    vector engine must never park on a swap semaphore, since the exp
    feed (mask adds, acc adds) queues behind it. The swaps ride the
    sync queue: gpsimd carries the hl1 denominator chain in segments."""
    nc = cx.nc
    tags = ("qsw", "ksw")
    sw = {}
    cx.__dict__.setdefault("_rot_sw", {})[(g, jp)] = sw

    def swap(src_i):
        def f():
            src = cx.qkv[g][src_i]
            sl = ds(jp * PAN, PAN)
            qsw = cx.swpool.tile([P, 2, PAN], BF16, tag=tags[src_i])
            nc.sync.dma_start(qsw[0:64, :, :], src[64:128, :, sl])
            nc.sync.dma_start(qsw[64:128, :, :], src[0:64, :, sl])
            sw[src_i] = qsw
        return f

    return [swap(0), swap(1)]


def _rot_dve_thunks(cx, g, jp):
    """DVE half of the doubled-angle rotary: q' = q*cos2 + swap(q)*sin2,
    in place on qT/kT panel jp."""
    nc = cx.nc

    def rot(src_i):
        def f():
            src = cx.qkv[g][src_i]
            sl = ds(jp * PAN, PAN)
            qsw = cx._rot_sw[(g, jp)][src_i]
            tmp = cx.rotpool.tile([P, 2, PAN], FP16, tag="rtmp", bufs=1)
            for hl in range(2):
                nc.vector.tensor_tensor(
                    tmp[:, hl, :], qsw[:, hl, :], cx.s2[:, sl], op=MULT
                )
            for hl in range(2):
                nc.vector.tensor_tensor(
                    src[:, hl, sl], src[:, hl, sl], cx.c2[:, sl], op=MULT
                )
            for hl in range(2):
                nc.vector.tensor_tensor(
                    src[:, hl, sl], src[:, hl, sl], tmp[:, hl, :], op=ADD
                )
        return f

    return [rot(0), rot(1)]


def _attn_thunks(cx, g, only_jp=None):
    """Thunk list for the attention of head pair g (heads 2g, 2g+1).

    Each t-panel's s-chunks run DIAGONAL-FIRST (the serialized masked
    chains overlap the following full-width chunks) and the QK and PV
    matmuls are software-pipelined with a one-chunk skew: PV(i) is
    emitted after QK(i+1), so every PV's exp latency is hidden by the
    next chunk's QK matmuls — the attention stream is self-covering
    and no longer relies on projection chains landing at panel tails.
    Finalize (denominator reduce + 1/Z) is deferred one chunk into the
    following panel."""
    nc = cx.nc
    thunks = []
    st8 = cx.__dict__.setdefault(f"_attn_state_{g}", {})
    exs = cx.__dict__.setdefault(f"_attn_ex_{g}", {})

    def qk_pair(jp, i, first):
        def f():
            qT, kT = cx.qkv[g]
            if first:
                for hl in range(2):
                    ytp = cx.psY.tile([P, PAN], F32, tag="psY")
                    acc = cx.accpool.tile([P, PAN], FP16, tag="acc")
                    st8[(hl, jp)] = (ytp, acc)
            di = i - 4 * jp
            off = P * di if di > 0 else 0
            w = PAN - off
            for hl in range(2):
                st = cx.psS.tile([P, PAN], F32, tag="psS")
                stw = st[:, off:PAN]
                nc.tensor.matmul(
                    stw,
                    lhsT=kT[:, hl, ds(i * P, P)],
                    rhs=qT[:, hl, ds(jp * PAN + off, w)],
                    start=True,
                    stop=True,
                )
                if di >= 0:
                    nc.vector.tensor_tensor(
                        st[:, off:off + P], st[:, off:off + P], cx.mk, op=ADD
                    )
                ex = cx.expool.tile([P, PAN], BF16, tag="ex")
                nc.scalar.activation(ex[:, off:PAN], stw, EXP, scale=SCALE)
                exs[(hl, jp, i)] = ex
        return f

    def pv_pair(jp, i, first, last):
        def f():
            di = i - 4 * jp
            off = P * di if di > 0 else 0
            for hl in range(2):
                ytp, acc = st8[(hl, jp)]
                ex = exs.pop((hl, jp, i))
                exw = ex[:, off:PAN]
                nc.tensor.matmul(
                    ytp[:, off:PAN],
                    lhsT=cx.v_sb[:, i, ds((2 * g + hl) * D, D)],
                    rhs=exw,
                    start=first,
                    stop=last,
                )
                # the two serial denominator chains run on separate
                # engines (hl0: vector, hl1: gpsimd) so the panel-tail
                # reduce never waits a single engine's backlog
                eng = nc.vector if hl == 0 else nc.gpsimd
                if first:
                    eng.tensor_copy(acc, ex)
                else:
                    eng.tensor_tensor(
                        acc[:, off:PAN], acc[:, off:PAN], exw, op=ADD
                    )
        return f

    def finalize(hl, jp):
        def f():
            h = 2 * g + hl
            ytp, acc = st8.pop((hl, jp))
            dps = cx.psS.tile([P, PAN], F32, tag="psS")
            nc.tensor.matmul(
                dps, lhsT=cx.onesmat, rhs=acc, start=True, stop=True
            )
            rdb = cx.dnpool.tile([P, PAN], F32, tag="rdb")
            nc.vector.reciprocal_approx_fast(out=rdb, in_=dps)
            nc.vector.tensor_tensor(
                cx.y_sb[:, h, ds(jp * PAN, PAN)], ytp, rdb, op=MULT
            )
        return f

    jps = range(NPAN) if only_jp is None else [only_jp]
    pending_fin = []
    for jp in jps:
        # diagonal chunks first, then the full-width history chunks
        order = list(range(4 * jp, 4 * jp + 4)) + list(range(4 * jp))
        for ci, i in enumerate(order):
            thunks.append(qk_pair(jp, i, ci == 0))
            if ci > 0:
                thunks.append(pv_pair(jp, order[ci - 1], ci == 1, False))
                if ci == 1 and pending_fin:
                    # previous panel's finalize lands one chunk into this
                    # panel, so its denominator matmul never heads the
                    # tensor queue while the acc chains drain
                    thunks.extend(pending_fin)
                    pending_fin = []
        thunks.append(pv_pair(jp, order[-1], len(order) == 1, True))
        pending_fin = [finalize(0, jp), finalize(1, jp)]
    thunks.extend(pending_fin)
    return thunks


def _outproj_thunks(cx, jp):
    """Output projection for t-panel jp over all 16 e-tiles. Output tiles
    are written bf16 and DMA'd on the sync/scalar queues so gpsimd can
    drain early."""
    nc = cx.nc
    thunks = []

    def etile(et):
        def f():
            ps = cx.psO.tile([P, PAN], F32, tag="psO")
            for dc in range(HPC):
                nc.tensor.matmul(
                    ps,
                    lhsT=cx.wo_sb[:, dc, ds(et * P, P)],
                    rhs=cx.y_sb[:, dc, ds(jp * PAN, PAN)],
                    start=(dc == 0),
                    stop=(dc == HPC - 1),
                )
            ob = cx.opool.tile([P, PAN], BF16, tag="ob")
            nc.scalar.copy(ob, ps)
            eng = cx.nc.sync if et % 2 == 0 else cx.nc.gpsimd
            eng.dma_start(
                cx.out[ds(et * P, P), ds(jp * PAN, PAN)], ob
            )
        return f

    for et in range(2 * HPC):
        thunks.append(etile(et))
    return thunks


def make_tables():
    j = np.arange(0, D, 2, dtype=np.float64) / D
    inv_freq = 1.0 / (10000.0 ** j)
    t = np.arange(T, dtype=np.float64)
    fr = np.outer(t, inv_freq)                            # [T, 64]
    c2 = np.cos(2.0 * fr).T                               # [64, T]
    s2 = np.sin(2.0 * fr).T
    cos2 = np.concatenate([c2, c2], axis=0).astype(np.float16)
    sin2 = np.concatenate([s2, -s2], axis=0).astype(np.float16)
    return cos2, sin2


def make_mask():
    import ml_dtypes
    s = np.arange(P)[:, None]
    c = np.arange(P)[None, :]
    return np.where(s <= c, 0.0, NEG).astype(ml_dtypes.bfloat16)


def make_in_maps(x, wq, wk, wv, wo):
    import ml_dtypes
    bf = ml_dtypes.bfloat16
    cos2, sin2 = make_tables()
    mask = make_mask()
    def _pair_eo(wT):
        # [E, 2D*G] -> group-major [g, eo2, p, two, d] -> [G*E/2, 4D]
        a = wT.reshape(E // (2 * P), 2, P, NGRP, 2 * D)
        a = a.transpose(3, 0, 2, 1, 4)          # [g, eo2, p, two, d]
        return np.ascontiguousarray(a.reshape(NGRP * (E // 2), 4 * D))

    in_maps = []
    for c in range(8):
        b, hh = c // 2, c % 2
        rows = slice(hh * HPC * D, (hh + 1) * HPC * D)
        in_maps.append({
            "xT": np.ascontiguousarray(x[b].T).astype(bf),
            "wqT": _pair_eo(np.ascontiguousarray(wq[rows].T)).astype(bf),
            "wkT": _pair_eo(np.ascontiguousarray(wk[rows].T)).astype(bf),
            "wvT": np.ascontiguousarray(wv[rows].T).astype(bf),
            "woT": np.ascontiguousarray(wo[:, rows].T).astype(bf),
            "cos2": cos2,
            "sin2": sin2,
            "mask": mask,
        })
    return in_maps


_PROGRAM_CACHE = {}


def get_program():
    if "nc" not in _PROGRAM_CACHE:
        _PROGRAM_CACHE["nc"] = build_program()
    return _PROGRAM_CACHE["nc"]


def kernel(x, wq, wk, wv, wo, _results_hook=None):
    x = np.asarray(x, dtype=np.float32)
    wq = np.asarray(wq, dtype=np.float32)
    wk = np.asarray(wk, dtype=np.float32)
    wv = np.asarray(wv, dtype=np.float32)
    wo = np.asarray(wo, dtype=np.float32)

    nc = get_program()
    in_maps = make_in_maps(x, wq, wk, wv, wo)
    res = run_bass_kernel_spmd(nc, in_maps, list(range(8)))
    if _results_hook is not None:
        _results_hook(res)
    outs = [np.asarray(r["out"]).astype(np.float32) for r in res.results]
    full = np.empty((B, T, E), dtype=np.float32)
    for b in range(B):
        full[b] = (outs[2 * b] + outs[2 * b + 1]).T
    return full
